# revision 10
# baseline (speedup 1.0000x reference)
"""4-layer GCN (ArithmeticCircuitGNN) on 8 Trainium2 NeuronCores.

Node-parallel: 12544-padded shard/core, LN affine folded into weights on
host, aggregation = AllGather(u) -> dma_gather(u[src]) -> dma_scatter_add
into V[dst], self-loops via V-init.  dma_scatter_add races on duplicate
rows within one call, so edges are packed into pieces (<=1024 idxs, the
SWDGE ring cap) with unique dsts per piece, spread over 2 V planes whose
piece chains serialize via Tile WAW while the planes run concurrently.

kernel(**inputs) takes FULL numpy inputs, returns FULL [100000,128] out.
"""

import os
import numpy as np
import ml_dtypes

import concourse.bass as bass
import concourse.bacc as bacc
import concourse.mybir as mybir
import concourse.tile as tile
from concourse.bass_utils import run_bass_kernel_spmd

BF16 = ml_dtypes.bfloat16

N = 100000
E = 300000
NCORES = 8
NLOC = 12500
NPAD = 12544          # 98 * 128
NT = 98
NG = NPAD * NCORES    # 100352
CHUNK = NPAD * 2      # 25088 rows (2 ranks) per gather chunk, < 32768
NCHUNK = 4
DUMMY = NPAD          # scatter dummy row; V has NPAD+128 rows
VROWS = NPAD + 128
D_IN, D_H, D_OUT = 128, 256, 128
EPS = 1e-5
PIECE = 1024          # max idxs per gather/scatter call (SWDGE ring cap)
NQ = 4                # SWDGE queues

F32 = mybir.dt.float32
BF = mybir.dt.bfloat16
I16 = mybir.dt.int16

SKIP_AGG = bool(int(os.environ.get("KERNEL_SKIP_AGG", "0")))
SKIP_CC = bool(int(os.environ.get("KERNEL_SKIP_CC", "0")))


# ---------------------------------------------------------------- host prep

def _wrap16(idx):
    """[M] -> [128, M//16]: position i -> (i%16, i//16), replicated x8."""
    M = len(idx)
    w = np.zeros((128, M // 16), dtype=np.int16)
    t = idx.reshape(M // 16, 16).T
    for g in range(8):
        w[g * 16:(g + 1) * 16, :] = t
    return w


def _prep_graph(edge_index):
    """Dup-free piece schedule.  Returns (gidx[8], sidx[8], schedule, M,
    dinv); schedule = [(plane, chunk, off, n), ...] uniform across cores."""
    src = np.asarray(edge_index[0], dtype=np.int64)
    dst = np.asarray(edge_index[1], dtype=np.int64)
    deg = np.bincount(dst, minlength=N).astype(np.float64) + 1.0
    dinv = (1.0 / np.sqrt(deg)).astype(np.float32)

    src_pid = (src // NLOC) * NPAD + (src % NLOC)

    per = [[[dict() for _ in range(NCHUNK)] for _ in range(2)]
           for _ in range(NCORES)]
    order = np.lexsort((src_pid, dst))
    s_sorted = src_pid[order]
    d_sorted = dst[order]
    bounds = np.flatnonzero(np.diff(d_sorted)) + 1
    starts = np.concatenate([[0], bounds])
    ends = np.concatenate([bounds, [len(d_sorted)]])
    for a, b in zip(starts, ends):
        d = int(d_sorted[a])
        r = d // NLOC
        dloc = d - r * NLOC
        for k, e in enumerate(range(a, b)):
            sp = int(s_sorted[e])
            c = sp // CHUNK
            per[r][k % 2][c].setdefault(dloc, []).append(sp - c * CHUNK)

    piece_contents = [[[None] * NCHUNK for _ in range(2)]
                      for _ in range(NCORES)]
    npieces = np.zeros((2, NCHUNK), dtype=np.int64)
    for r in range(NCORES):
        for p in range(2):
            for c in range(NCHUNK):
                groups = per[r][p][c]
                n = sum(len(v) for v in groups.values())
                k = max(1, -(-n // (PIECE - 16)))
                if groups:
                    k = max(k, max(len(v) for v in groups.values()))
                buckets = [[] for _ in range(k)]
                for dloc, srcs in sorted(groups.items(),
                                         key=lambda kv: -len(kv[1])):
                    bs = sorted(range(k), key=lambda i: len(buckets[i]))
                    for i, sp in enumerate(srcs):
                        buckets[bs[i]].append((sp, dloc))
                assert max(len(b) for b in buckets) <= PIECE
                piece_contents[r][p][c] = buckets
                npieces[p, c] = max(npieces[p, c], k)

    sizes = {}
    for p in range(2):
        for c in range(NCHUNK):
            for i in range(npieces[p, c]):
                mx = 1
                for r in range(NCORES):
                    bl = piece_contents[r][p][c]
                    if i < len(bl):
                        mx = max(mx, len(bl[i]))
                sizes[(p, c, i)] = -(-mx // 128) * 128

    schedule = []
    idx_of = {}
    off = 0
    for p in range(2):
        for c in range(NCHUNK):
            for i in range(npieces[p, c]):
                n = sizes[(p, c, i)]
                idx_of[off] = i
                schedule.append((p, c, off, n))
                off += n
    M = off

    gidx, sidx = [], []
    for r in range(NCORES):
        g = np.zeros(M, dtype=np.int16)
        sx = np.full(M, DUMMY, dtype=np.int16)
        for (p, c, o, n) in schedule:
            i = idx_of[o]
            bl = piece_contents[r][p][c]
            if i < len(bl):
                for j, (sp, dloc) in enumerate(bl[i]):
                    g[o + j] = sp
                    sx[o + j] = dloc
        gidx.append(_wrap16(g))
        sidx.append(_wrap16(sx))
    return gidx, sidx, schedule, M, dinv


def _pad_nodes(a):
    d = a.shape[1]
    out = np.zeros((NG, d), dtype=a.dtype)
    for r in range(NCORES):
        out[r * NPAD:r * NPAD + NLOC] = a[r * NLOC:(r + 1) * NLOC]
    return out


def _rep(v, p=128):
    return np.ascontiguousarray(
        np.broadcast_to(np.asarray(v, np.float32), (p, len(v))))


# ---------------------------------------------------------------- builder

def _build(M, schedule, use_brow1, use_brow2, use_fg, use_fb):
    nc = bacc.Bacc(None, target_bir_lowering=False, num_swdge_queues=NQ)

    def param(name, shape, dt, out=False):
        return nc.declare_dram_parameter(name, shape, dt, isOutput=out)

    u0_own = param("u0_own", [NPAD, D_IN], F32)
    gidx = param("gidx", [128, M // 16], I16)
    sidx = param("sidx", [128, M // 16], I16)
    dinv_p = param("dinv", [128, NT], F32)
    ident_p = param("ident", [128, 128], BF)
    w0_p = param("w0", [128, D_H], BF)
    w1_p = param("w1", [2, 128, D_H], BF)
    w2_p = param("w2", [2, 128, D_H], BF)
    w3_p = param("w3", [2, 128, D_OUT], BF)
    b0_p = param("b0r", [128, D_H], F32)
    b1_p = param("b1r", [128, D_H], F32)
    b2_p = param("b2r", [128, D_H], F32)
    b3_p = param("b3r", [128, D_OUT], F32)
    brow1_p = param("brow1r", [128, D_H], F32) if use_brow1 else None
    brow2_p = param("brow2r", [128, D_H], F32) if use_brow2 else None
    fg_p = param("fgr", [128, D_OUT], F32) if use_fg else None
    fb_p = param("fbr", [128, D_OUT], F32) if use_fb else None
    out_p = param("out", [NLOC, D_OUT], F32, out=True)

    VA0 = nc.dram_tensor("VA0", [VROWS, D_IN], F32)
    VB0 = nc.dram_tensor("VB0", [VROWS, D_IN], F32)
    VA23 = nc.dram_tensor("VA23", [VROWS, D_H], BF)
    VB23 = nc.dram_tensor("VB23", [VROWS, D_H], BF)
    VA4 = nc.dram_tensor("VA4", [VROWS, D_OUT], F32)
    VB4 = nc.dram_tensor("VB4", [VROWS, D_OUT], F32)
    ul0 = nc.dram_tensor("ul0", [NPAD, D_IN], F32)
    uf0 = nc.dram_tensor("uf0", [NG, D_IN], F32, addr_space="Shared")
    ul23 = nc.dram_tensor("ul23", [NPAD, D_H], BF)
    uf23 = nc.dram_tensor("uf23", [NG, D_H], BF, addr_space="Shared")
    ul4 = nc.dram_tensor("ul4", [NPAD, D_OUT], F32)
    uf4 = nc.dram_tensor("uf4", [NG, D_OUT], F32, addr_space="Shared")

    AX = mybir.AxisListType.X
    AF = mybir.ActivationFunctionType
    OP = mybir.AluOpType

    with tile.TileContext(nc) as tc:
        with (
            tc.tile_pool(name="const", bufs=1) as cp,
            tc.tile_pool(name="hbuf", bufs=1) as hp,
            tc.tile_pool(name="work", bufs=3) as wp,
            tc.tile_pool(name="small", bufs=4) as sp,
            tc.tile_pool(name="msg", bufs=4) as mp,
            tc.tile_pool(name="psT", bufs=2, space="PSUM") as pT,
            tc.tile_pool(name="psM", bufs=2, space="PSUM") as pM,
        ):
            def cload(par, shape, dt):
                t = cp.tile(shape, dt, tag=par.name)
                nc.sync.dma_start(t[:], par[:])
                return t

            gi = cload(gidx, [128, M // 16], I16)
            si = cload(sidx, [128, M // 16], I16)
            dv = cload(dinv_p, [128, NT], F32)
            idn = cload(ident_p, [128, 128], BF)
            w0 = cload(w0_p, [128, D_H], BF)

            def wload(par, d):
                t = cp.tile([128, 2, d], BF, tag=par.name)
                nc.sync.dma_start(t[:], par.rearrange("k p d -> p k d"))
                return t

            w1 = wload(w1_p, D_H)
            w2 = wload(w2_p, D_H)
            w3 = wload(w3_p, D_OUT)
            b0 = cload(b0_p, [128, D_H], F32)
            b1 = cload(b1_p, [128, D_H], F32)
            b2 = cload(b2_p, [128, D_H], F32)
            b3 = cload(b3_p, [128, D_OUT], F32)
            brow1 = cload(brow1_p, [128, D_H], F32) if use_brow1 else None
            brow2 = cload(brow2_p, [128, D_H], F32) if use_brow2 else None
            fg = cload(fg_p, [128, D_OUT], F32) if use_fg else None
            fb = cload(fb_p, [128, D_OUT], F32) if use_fb else None

            zbf = cp.tile([128, 14, D_H], BF, tag="zbf")
            nc.vector.memset(zbf[:], 0.0)
            zf = cp.tile([128, 14, D_IN], F32, tag="zf")
            nc.vector.memset(zf[:], 0.0)

            h_sb = hp.tile([128, NT, D_H], BF)

            def r3(t, d):
                return t.rearrange("(n p) d -> p n d", p=128)

            def zero_plane(V, d, zt):
                for g in range(7):
                    nc.sync.dma_start(
                        r3(V[0:NPAD], d)[:, g * 14:(g + 1) * 14, :], zt[:])

            def aggregate(uf, Va, Vb, d, dt):
                """V[plane][sidx] += uf[gidx] over the piece schedule."""
                if SKIP_AGG:
                    return
                for j, (p, c, off, n) in enumerate(schedule):
                    mt = mp.tile([128, PIECE // 128, d], dt, tag="msg")
                    src = uf[c * CHUNK:(c + 1) * CHUNK, :]
                    q = j % NQ
                    nc.gpsimd.dma_gather(
                        mt[:, : n // 128, :], src,
                        gi[:, off // 16:(off + n) // 16], n, n, d,
                        queue_num=q,
                    )
                    V = Va if p == 0 else Vb
                    nc.gpsimd.dma_scatter_add(
                        V[:, :], mt[:, : n // 128, :],
                        si[:, off // 16:(off + n) // 16], n, n, d,
                        queue_num=q,
                    )

            def allgather(ul, uf):
                if SKIP_CC:
                    return
                nc.gpsimd.collective_compute(
                    "AllGather", OP.bypass,
                    ins=[ul[:].opt()], outs=[uf[:].opt()],
                    replica_groups=[list(range(NCORES))],
                )

            def transpose_mm(z_bf, w, d_out, kchunks):
                mm = pM.tile([128, d_out], F32, tag="mm")
                for k in range(kchunks):
                    zt_ps = pT.tile([128, 128], BF, tag="zt_ps")
                    nc.tensor.transpose(
                        zt_ps[:], z_bf[:, k * 128:(k + 1) * 128], idn[:])
                    zt = wp.tile([128, 128], BF, tag="zt")
                    nc.scalar.activation(zt[:], zt_ps[:], AF.Copy)
                    nc.tensor.matmul(mm[:], zt[:],
                                     w[:, k, :] if kchunks > 1 else w[:],
                                     start=(k == 0), stop=(k == kchunks - 1))
                return mm

            # ================= Layer 1 ====================================
            for g in range(7):
                t0 = mp.tile([128, 14, D_IN], F32, tag="v0i")
                nc.sync.dma_start(
                    t0[:], r3(u0_own, D_IN)[:, g * 14:(g + 1) * 14, :])
                nc.sync.dma_start(
                    r3(VA0[0:NPAD], D_IN)[:, g * 14:(g + 1) * 14, :], t0[:])
                nc.sync.dma_start(
                    r3(ul0, D_IN)[:, g * 14:(g + 1) * 14, :], t0[:])
            zero_plane(VB0, D_IN, zf)
            allgather(ul0, uf0)
            aggregate(uf0, VA0, VB0, D_IN, F32)
            for b in range(NT):
                vA = wp.tile([128, D_IN], F32, tag="vA")
                nc.sync.dma_start(vA[:], r3(VA0[0:NPAD], D_IN)[:, b, :])
                vB = wp.tile([128, D_IN], F32, tag="vB")
                nc.sync.dma_start(vB[:], r3(VB0[0:NPAD], D_IN)[:, b, :])
                t = wp.tile([128, D_IN], F32, tag="t")
                nc.vector.tensor_tensor(t[:], vA[:], vB[:], OP.add)
                agg = wp.tile([128, D_IN], BF, tag="z")
                nc.vector.tensor_scalar_mul(agg[:], t[:], dv[:, b:b + 1])
                mm = transpose_mm(agg, w0, D_H, 1)
                t2 = wp.tile([128, D_H], F32, tag="u")
                nc.vector.tensor_tensor(t2[:], mm[:], b0[:], OP.add)
                nc.scalar.activation(h_sb[:, b, :], t2[:], AF.Relu)

            # ================= Layers 2,3 =================================
            for li, (w, bias, brow) in enumerate(
                    [(w1, b1, brow1), (w2, b2, brow2)]):
                for b in range(NT):
                    ht = h_sb[:, b, :]
                    sums = sp.tile([128, 1], F32, tag="sums")
                    nc.vector.tensor_reduce(sums[:], ht, AX, OP.add)
                    negmu = sp.tile([128, 1], F32, tag="negmu")
                    nc.vector.tensor_scalar_mul(negmu[:], sums[:], -1.0 / D_H)
                    sq = wp.tile([128, D_H], F32, tag="sq")
                    ssq = sp.tile([128, 1], F32, tag="ssq")
                    nc.scalar.activation(sq[:], ht, AF.Square, bias=negmu[:],
                                         accum_out=ssq[:])
                    varp = sp.tile([128, 1], F32, tag="varp")
                    nc.vector.tensor_scalar(varp[:], ssq[:], 1.0 / D_H, EPS,
                                            OP.mult, OP.add)
                    sd = sp.tile([128, 1], F32, tag="sd")
                    nc.scalar.sqrt(sd[:], varp[:])
                    rstd = sp.tile([128, 1], F32, tag="rstd")
                    nc.vector.reciprocal(rstd[:], sd[:])
                    s = sp.tile([128, 1], F32, tag="s")
                    nc.vector.tensor_tensor(s[:], rstd[:], dv[:, b:b + 1],
                                            OP.mult)
                    z = wp.tile([128, D_H], BF, tag="z")
                    nc.vector.tensor_scalar(z[:], ht, negmu[:], s[:],
                                            OP.add, OP.mult)
                    mm = transpose_mm(z, w, D_H, 2)
                    u = wp.tile([128, D_H], BF, tag="u")
                    if brow is not None:
                        nc.vector.scalar_tensor_tensor(
                            u[:], brow[:], dv[:, b:b + 1], mm[:],
                            OP.mult, OP.add)
                    else:
                        nc.scalar.activation(u[:], mm[:], AF.Copy)
                    nc.sync.dma_start(r3(ul23, D_H)[:, b, :], u[:])
                    nc.sync.dma_start(r3(VA23[0:NPAD], D_H)[:, b, :], u[:])
                zero_plane(VB23, D_H, zbf)
                allgather(ul23, uf23)
                aggregate(uf23, VA23, VB23, D_H, BF)
                for b in range(NT):
                    vA = wp.tile([128, D_H], BF, tag="vA")
                    nc.sync.dma_start(vA[:], r3(VA23[0:NPAD], D_H)[:, b, :])
                    vB = wp.tile([128, D_H], BF, tag="vB")
                    nc.sync.dma_start(vB[:], r3(VB23[0:NPAD], D_H)[:, b, :])
                    t = wp.tile([128, D_H], F32, tag="t")
                    nc.vector.scalar_tensor_tensor(
                        t[:], vB[:], dv[:, b:b + 1], bias[:], OP.mult, OP.add)
                    t2 = wp.tile([128, D_H], F32, tag="t2")
                    nc.vector.scalar_tensor_tensor(
                        t2[:], vA[:], dv[:, b:b + 1], t[:], OP.mult, OP.add)
                    r = wp.tile([128, D_H], F32, tag="r")
                    nc.scalar.activation(r[:], t2[:], AF.Relu)
                    nc.vector.tensor_tensor(h_sb[:, b, :], r[:],
                                            h_sb[:, b, :], OP.add)

            # ================= Layer 4 ====================================
            for b in range(NT):
                z = wp.tile([128, D_H], BF, tag="z")
                nc.vector.tensor_scalar_mul(z[:], h_sb[:, b, :],
                                            dv[:, b:b + 1])
                mm = transpose_mm(z, w3, D_OUT, 2)
                u = wp.tile([128, D_OUT], F32, tag="u")
                nc.scalar.activation(u[:], mm[:], AF.Copy)
                nc.sync.dma_start(r3(ul4, D_OUT)[:, b, :], u[:])
                nc.sync.dma_start(r3(VA4[0:NPAD], D_OUT)[:, b, :], u[:])
            zero_plane(VB4, D_OUT, zf)
            allgather(ul4, uf4)
            aggregate(uf4, VA4, VB4, D_OUT, F32)
            for b in range(NT):
                vA = wp.tile([128, D_OUT], F32, tag="vA")
                nc.sync.dma_start(vA[:], r3(VA4[0:NPAD], D_OUT)[:, b, :])
                vB = wp.tile([128, D_OUT], F32, tag="vB")
                nc.sync.dma_start(vB[:], r3(VB4[0:NPAD], D_OUT)[:, b, :])
                y = wp.tile([128, D_OUT], F32, tag="t")
                nc.vector.scalar_tensor_tensor(
                    y[:], vB[:], dv[:, b:b + 1], b3[:], OP.mult, OP.add)
                y2 = wp.tile([128, D_OUT], F32, tag="t2")
                nc.vector.scalar_tensor_tensor(
                    y2[:], vA[:], dv[:, b:b + 1], y[:], OP.mult, OP.add)
                sums = sp.tile([128, 1], F32, tag="sums")
                nc.vector.tensor_reduce(sums[:], y2[:], AX, OP.add)
                negmu = sp.tile([128, 1], F32, tag="negmu")
                nc.vector.tensor_scalar_mul(negmu[:], sums[:], -1.0 / D_OUT)
                sq = wp.tile([128, D_OUT], F32, tag="sq")
                ssq = sp.tile([128, 1], F32, tag="ssq")
                nc.scalar.activation(sq[:], y2[:], AF.Square, bias=negmu[:],
                                     accum_out=ssq[:])
                varp = sp.tile([128, 1], F32, tag="varp")
                nc.vector.tensor_scalar(varp[:], ssq[:], 1.0 / D_OUT, EPS,
                                        OP.mult, OP.add)
                sd = sp.tile([128, 1], F32, tag="sd")
                nc.scalar.sqrt(sd[:], varp[:])
                rstd = sp.tile([128, 1], F32, tag="rstd")
                nc.vector.reciprocal(rstd[:], sd[:])
                zo = wp.tile([128, D_OUT], F32, tag="r")
                nc.vector.tensor_scalar(zo[:], y2[:], negmu[:], rstd[:],
                                        OP.add, OP.mult)
                if fg is not None:
                    zo2 = wp.tile([128, D_OUT], F32, tag="zo2")
                    nc.vector.tensor_tensor(zo2[:], zo[:], fg[:], OP.mult)
                    zo = zo2
                if fb is not None:
                    zo3 = wp.tile([128, D_OUT], F32, tag="zo3")
                    nc.vector.tensor_tensor(zo3[:], zo[:], fb[:], OP.add)
                    zo = zo3
                lo = b * 128
                nrow = min(128, NLOC - lo)
                if nrow > 0:
                    nc.sync.dma_start(out_p[lo:lo + nrow, :], zo[0:nrow, :])

    nc.compile()
    return nc


_CACHE = {}


def kernel(x, edge_index, W0, b0, W1, b1, W2, b2, W3, b3,
           ln0_g, ln0_b, ln1_g, ln1_b, fln_g, fln_b):
    x = np.asarray(x, np.float32)
    edge_index = np.asarray(edge_index)
    gidx, sidx, schedule, M, dinv = _prep_graph(edge_index)

    W1f = np.asarray(ln0_g, np.float32)[:, None] * np.asarray(W1, np.float32)
    W2f = np.asarray(ln1_g, np.float32)[:, None] * np.asarray(W2, np.float32)
    brow1 = np.asarray(ln0_b, np.float32) @ np.asarray(W1, np.float32)
    brow2 = np.asarray(ln1_b, np.float32) @ np.asarray(W2, np.float32)
    use_brow1 = bool(np.any(brow1 != 0))
    use_brow2 = bool(np.any(brow2 != 0))
    use_fg = bool(np.any(np.asarray(fln_g) != 1))
    use_fb = bool(np.any(np.asarray(fln_b) != 0))

    key = (M, tuple(schedule), use_brow1, use_brow2, use_fg, use_fb)
    if key not in _CACHE:
        _CACHE[key] = _build(M, schedule, use_brow1, use_brow2,
                             use_fg, use_fb)
    nc = _CACHE[key]

    u0 = _pad_nodes(dinv[:, None].astype(np.float32) * x)
    dinv_pad = np.zeros((NCORES, NPAD), np.float32)
    for r in range(NCORES):
        dinv_pad[r, :NLOC] = dinv[r * NLOC:(r + 1) * NLOC]

    def chunk2(Wf):
        return np.stack([Wf[0:128], Wf[128:256]]).astype(BF16)

    common = {
        "ident": np.eye(128, dtype=BF16),
        "w0": np.asarray(W0, np.float32).astype(BF16),
        "w1": chunk2(W1f), "w2": chunk2(W2f),
        "w3": chunk2(np.asarray(W3, np.float32)),
        "b0r": _rep(b0), "b1r": _rep(b1), "b2r": _rep(b2), "b3r": _rep(b3),
    }
    if use_brow1:
        common["brow1r"] = _rep(brow1)
    if use_brow2:
        common["brow2r"] = _rep(brow2)
    if use_fg:
        common["fgr"] = _rep(fln_g)
    if use_fb:
        common["fbr"] = _rep(fln_b)

    in_maps = []
    for r in range(NCORES):
        m = dict(common)
        m["u0_own"] = u0[r * NPAD:(r + 1) * NPAD]
        m["gidx"] = gidx[r]
        m["sidx"] = sidx[r]
        m["dinv"] = np.ascontiguousarray(dinv_pad[r].reshape(NT, 128).T)
        in_maps.append(m)

    res = run_bass_kernel_spmd(nc, in_maps, core_ids=list(range(NCORES)))
    out = np.concatenate([res.results[r]["out"] for r in range(NCORES)],
                         axis=0)
    return out.astype(np.float32)


# revision 12
# speedup vs baseline: 1.0982x; 1.0982x over previous
"""4-layer GCN (ArithmeticCircuitGNN) on 8 Trainium2 NeuronCores.

Node-parallel: 12544-padded shard/core, LN affine folded into weights on
host, aggregation = AllGather(u) -> dma_gather(u[src]) -> dma_scatter_add
into V[dst], self-loops via V-init.  dma_scatter_add races on duplicate
rows within one call, so edges are packed into pieces (<=1024 idxs, the
SWDGE ring cap) with unique dsts per piece, spread over 2 V planes whose
piece chains serialize via Tile WAW while the planes run concurrently.

kernel(**inputs) takes FULL numpy inputs, returns FULL [100000,128] out.
"""

import os
import numpy as np
import ml_dtypes

import concourse.bass as bass
import concourse.bacc as bacc
import concourse.mybir as mybir
import concourse.tile as tile
from concourse.bass_utils import run_bass_kernel_spmd

BF16 = ml_dtypes.bfloat16

N = 100000
E = 300000
NCORES = 8
NLOC = 12500
NPAD = 12544          # 98 * 128
NT = 98
NG = NPAD * NCORES    # 100352
CHUNK = NPAD * 2      # 25088 rows (2 ranks) per gather chunk, < 32768
NCHUNK = 4
DUMMY = NPAD          # scatter dummy row; V has NPAD+128 rows
VROWS = NPAD + 128
D_IN, D_H, D_OUT = 128, 256, 128
EPS = 1e-5
PIECE = 1024          # max idxs per gather/scatter call (SWDGE ring cap)
NQ = 4                # SWDGE queues

F32 = mybir.dt.float32
BF = mybir.dt.bfloat16
I16 = mybir.dt.int16

SKIP_AGG = bool(int(os.environ.get("KERNEL_SKIP_AGG", "0")))
SKIP_CC = bool(int(os.environ.get("KERNEL_SKIP_CC", "0")))


# ---------------------------------------------------------------- host prep

def _wrap16(idx):
    """[M] -> [128, M//16]: position i -> (i%16, i//16), replicated x8."""
    M = len(idx)
    w = np.zeros((128, M // 16), dtype=np.int16)
    t = idx.reshape(M // 16, 16).T
    for g in range(8):
        w[g * 16:(g + 1) * 16, :] = t
    return w


def _prep_graph(edge_index):
    """Dup-free piece schedule.  Returns (gidx[8], sidx[8], schedule, M,
    dinv); schedule = [(plane, chunk, off, n), ...] uniform across cores."""
    src = np.asarray(edge_index[0], dtype=np.int64)
    dst = np.asarray(edge_index[1], dtype=np.int64)
    deg = np.bincount(dst, minlength=N).astype(np.float64) + 1.0
    dinv = (1.0 / np.sqrt(deg)).astype(np.float32)

    src_pid = (src // NLOC) * NPAD + (src % NLOC)

    per = [[[dict() for _ in range(NCHUNK)] for _ in range(2)]
           for _ in range(NCORES)]
    order = np.lexsort((src_pid, dst))
    s_sorted = src_pid[order]
    d_sorted = dst[order]
    bounds = np.flatnonzero(np.diff(d_sorted)) + 1
    starts = np.concatenate([[0], bounds])
    ends = np.concatenate([bounds, [len(d_sorted)]])
    for a, b in zip(starts, ends):
        d = int(d_sorted[a])
        r = d // NLOC
        dloc = d - r * NLOC
        for k, e in enumerate(range(a, b)):
            sp = int(s_sorted[e])
            c = sp // CHUNK
            per[r][k % 2][c].setdefault(dloc, []).append(sp - c * CHUNK)

    piece_contents = [[[None] * NCHUNK for _ in range(2)]
                      for _ in range(NCORES)]
    npieces = np.zeros((2, NCHUNK), dtype=np.int64)
    for r in range(NCORES):
        for p in range(2):
            for c in range(NCHUNK):
                groups = per[r][p][c]
                n = sum(len(v) for v in groups.values())
                k = max(1, -(-n // (PIECE - 16)))
                if groups:
                    k = max(k, max(len(v) for v in groups.values()))
                buckets = [[] for _ in range(k)]
                for dloc, srcs in sorted(groups.items(),
                                         key=lambda kv: -len(kv[1])):
                    bs = sorted(range(k), key=lambda i: len(buckets[i]))
                    for i, sp in enumerate(srcs):
                        buckets[bs[i]].append((sp, dloc))
                assert max(len(b) for b in buckets) <= PIECE
                piece_contents[r][p][c] = buckets
                npieces[p, c] = max(npieces[p, c], k)

    sizes = {}
    for p in range(2):
        for c in range(NCHUNK):
            for i in range(npieces[p, c]):
                mx = 1
                for r in range(NCORES):
                    bl = piece_contents[r][p][c]
                    if i < len(bl):
                        mx = max(mx, len(bl[i]))
                sizes[(p, c, i)] = -(-mx // 128) * 128

    per_plane = [[], []]
    for p in range(2):
        for c in range(NCHUNK):
            for i in range(npieces[p, c]):
                per_plane[p].append((p, c, i))
    merged = []
    a, bq = per_plane
    for i in range(max(len(a), len(bq))):
        if i < len(a):
            merged.append(a[i])
        if i < len(bq):
            merged.append(bq[i])
    schedule = []
    idx_of = {}
    off = 0
    for (p, c, i) in merged:
        n = sizes[(p, c, i)]
        idx_of[off] = i
        schedule.append((p, c, off, n))
        off += n
    M = off

    gidx, sidx = [], []
    for r in range(NCORES):
        g = np.zeros(M, dtype=np.int16)
        sx = np.full(M, DUMMY, dtype=np.int16)
        for (p, c, o, n) in schedule:
            i = idx_of[o]
            bl = piece_contents[r][p][c]
            if i < len(bl):
                for j, (sp, dloc) in enumerate(bl[i]):
                    g[o + j] = sp
                    sx[o + j] = dloc
        gidx.append(_wrap16(g))
        sidx.append(_wrap16(sx))
    return gidx, sidx, schedule, M, dinv


def _pad_nodes(a):
    d = a.shape[1]
    out = np.zeros((NG, d), dtype=a.dtype)
    for r in range(NCORES):
        out[r * NPAD:r * NPAD + NLOC] = a[r * NLOC:(r + 1) * NLOC]
    return out


def _rep(v, p=128):
    return np.ascontiguousarray(
        np.broadcast_to(np.asarray(v, np.float32), (p, len(v))))


# ---------------------------------------------------------------- builder

def _build(M, schedule, use_brow1, use_brow2, use_fg, use_fb):
    nc = bacc.Bacc(None, target_bir_lowering=False, num_swdge_queues=NQ)

    def param(name, shape, dt, out=False):
        return nc.declare_dram_parameter(name, shape, dt, isOutput=out)

    u0_own = param("u0_own", [NPAD, D_IN], F32)
    gidx = param("gidx", [128, M // 16], I16)
    sidx = param("sidx", [128, M // 16], I16)
    dinv_p = param("dinv", [128, NT], F32)
    ident_p = param("ident", [128, 128], BF)
    w0_p = param("w0", [128, D_H], BF)
    w1_p = param("w1", [2, 128, D_H], BF)
    w2_p = param("w2", [2, 128, D_H], BF)
    w3_p = param("w3", [2, 128, D_OUT], BF)
    b0_p = param("b0r", [128, D_H], F32)
    b1_p = param("b1r", [128, D_H], F32)
    b2_p = param("b2r", [128, D_H], F32)
    b3_p = param("b3r", [128, D_OUT], F32)
    brow1_p = param("brow1r", [128, D_H], F32) if use_brow1 else None
    brow2_p = param("brow2r", [128, D_H], F32) if use_brow2 else None
    fg_p = param("fgr", [128, D_OUT], F32) if use_fg else None
    fb_p = param("fbr", [128, D_OUT], F32) if use_fb else None
    out_p = param("out", [NLOC, D_OUT], F32, out=True)

    VA0 = nc.dram_tensor("VA0", [VROWS, D_IN], F32)
    VB0 = nc.dram_tensor("VB0", [VROWS, D_IN], F32)
    VA23 = nc.dram_tensor("VA23", [VROWS, D_H], BF)
    VB23 = nc.dram_tensor("VB23", [VROWS, D_H], BF)
    VA4 = nc.dram_tensor("VA4", [VROWS, D_OUT], F32)
    VB4 = nc.dram_tensor("VB4", [VROWS, D_OUT], F32)
    ul0 = nc.dram_tensor("ul0", [NPAD, D_IN], F32)
    uf0 = nc.dram_tensor("uf0", [NG, D_IN], F32, addr_space="Shared")
    ul23 = nc.dram_tensor("ul23", [NPAD, D_H], BF)
    uf23 = nc.dram_tensor("uf23", [NG, D_H], BF, addr_space="Shared")
    ul4 = nc.dram_tensor("ul4", [NPAD, D_OUT], F32)
    uf4 = nc.dram_tensor("uf4", [NG, D_OUT], F32, addr_space="Shared")

    AX = mybir.AxisListType.X
    AF = mybir.ActivationFunctionType
    OP = mybir.AluOpType

    with tile.TileContext(nc) as tc:
        with (
            tc.tile_pool(name="const", bufs=1) as cp,
            tc.tile_pool(name="hbuf", bufs=1) as hp,
            tc.tile_pool(name="work", bufs=3) as wp,
            tc.tile_pool(name="small", bufs=4) as sp,
            tc.tile_pool(name="msg", bufs=6) as mp,
            tc.tile_pool(name="psT", bufs=2, space="PSUM") as pT,
            tc.tile_pool(name="psM", bufs=2, space="PSUM") as pM,
        ):
            def cload(par, shape, dt):
                t = cp.tile(shape, dt, tag=par.name)
                nc.sync.dma_start(t[:], par[:])
                return t

            gi = cload(gidx, [128, M // 16], I16)
            si = cload(sidx, [128, M // 16], I16)
            dv = cload(dinv_p, [128, NT], F32)
            idn = cload(ident_p, [128, 128], BF)
            w0 = cload(w0_p, [128, D_H], BF)

            def wload(par, d):
                t = cp.tile([128, 2, d], BF, tag=par.name)
                nc.sync.dma_start(t[:], par.rearrange("k p d -> p k d"))
                return t

            w1 = wload(w1_p, D_H)
            w2 = wload(w2_p, D_H)
            w3 = wload(w3_p, D_OUT)
            b0 = cload(b0_p, [128, D_H], F32)
            b1 = cload(b1_p, [128, D_H], F32)
            b2 = cload(b2_p, [128, D_H], F32)
            b3 = cload(b3_p, [128, D_OUT], F32)
            brow1 = cload(brow1_p, [128, D_H], F32) if use_brow1 else None
            brow2 = cload(brow2_p, [128, D_H], F32) if use_brow2 else None
            fg = cload(fg_p, [128, D_OUT], F32) if use_fg else None
            fb = cload(fb_p, [128, D_OUT], F32) if use_fb else None

            zbf = cp.tile([128, 14, D_H], BF, tag="zbf")
            nc.vector.memset(zbf[:], 0.0)
            zf = cp.tile([128, 14, D_IN], F32, tag="zf")
            nc.vector.memset(zf[:], 0.0)

            h_sb = hp.tile([128, NT, D_H], BF)

            def r3(t, d):
                return t.rearrange("(n p) d -> p n d", p=128)

            def zero_plane(V, d, zt):
                for g in range(7):
                    nc.sync.dma_start(
                        r3(V[0:NPAD], d)[:, g * 14:(g + 1) * 14, :], zt[:])

            def aggregate(uf, Va, Vb, d, dt):
                """V[plane][sidx] += uf[gidx] over the piece schedule.

                Gathers are emitted LOOKAHEAD pieces ahead of their scatters
                so the scatter's wait (on its gather's DMA + its plane's WAW
                chain) doesn't head-of-line-block desc-gen on the Q7."""
                if SKIP_AGG:
                    return
                LOOKAHEAD = 3
                mts = {}

                def emit_gather(j):
                    (p, c, off, n) = schedule[j]
                    mt = mp.tile([128, PIECE // 128, d], dt, tag="msg")
                    src = uf[c * CHUNK:(c + 1) * CHUNK, :]
                    nc.gpsimd.dma_gather(
                        mt[:, : n // 128, :], src,
                        gi[:, off // 16:(off + n) // 16], n, n, d,
                        queue_num=j % NQ,
                    )
                    mts[j] = mt

                def emit_scatter(j):
                    (p, c, off, n) = schedule[j]
                    V = Va if p == 0 else Vb
                    nc.gpsimd.dma_scatter_add(
                        V[:, :], mts.pop(j)[:, : n // 128, :],
                        si[:, off // 16:(off + n) // 16], n, n, d,
                        queue_num=j % NQ,
                    )

                for j in range(len(schedule)):
                    emit_gather(j)
                    if j >= LOOKAHEAD:
                        emit_scatter(j - LOOKAHEAD)
                for j in range(len(schedule) - LOOKAHEAD, len(schedule)):
                    emit_scatter(j)

            def allgather(ul, uf):
                if SKIP_CC:
                    return
                nc.gpsimd.collective_compute(
                    "AllGather", OP.bypass,
                    ins=[ul[:].opt()], outs=[uf[:].opt()],
                    replica_groups=[list(range(NCORES))],
                )

            def transpose_mm(z_bf, w, d_out, kchunks):
                mm = pM.tile([128, d_out], F32, tag="mm")
                for k in range(kchunks):
                    zt_ps = pT.tile([128, 128], BF, tag="zt_ps")
                    nc.tensor.transpose(
                        zt_ps[:], z_bf[:, k * 128:(k + 1) * 128], idn[:])
                    zt = wp.tile([128, 128], BF, tag="zt")
                    nc.scalar.activation(zt[:], zt_ps[:], AF.Copy)
                    nc.tensor.matmul(mm[:], zt[:],
                                     w[:, k, :] if kchunks > 1 else w[:],
                                     start=(k == 0), stop=(k == kchunks - 1))
                return mm

            # ================= Layer 1 ====================================
            for g in range(7):
                t0 = mp.tile([128, 14, D_IN], F32, tag="v0i")
                nc.sync.dma_start(
                    t0[:], r3(u0_own, D_IN)[:, g * 14:(g + 1) * 14, :])
                nc.sync.dma_start(
                    r3(VA0[0:NPAD], D_IN)[:, g * 14:(g + 1) * 14, :], t0[:])
                nc.sync.dma_start(
                    r3(ul0, D_IN)[:, g * 14:(g + 1) * 14, :], t0[:])
            zero_plane(VB0, D_IN, zf)
            allgather(ul0, uf0)
            aggregate(uf0, VA0, VB0, D_IN, F32)
            for b in range(NT):
                vA = wp.tile([128, D_IN], F32, tag="vA")
                nc.sync.dma_start(vA[:], r3(VA0[0:NPAD], D_IN)[:, b, :])
                vB = wp.tile([128, D_IN], F32, tag="vB")
                nc.sync.dma_start(vB[:], r3(VB0[0:NPAD], D_IN)[:, b, :])
                t = wp.tile([128, D_IN], F32, tag="t")
                nc.vector.tensor_tensor(t[:], vA[:], vB[:], OP.add)
                agg = wp.tile([128, D_IN], BF, tag="z")
                nc.vector.tensor_scalar_mul(agg[:], t[:], dv[:, b:b + 1])
                mm = transpose_mm(agg, w0, D_H, 1)
                t2 = wp.tile([128, D_H], F32, tag="u")
                nc.vector.tensor_tensor(t2[:], mm[:], b0[:], OP.add)
                nc.scalar.activation(h_sb[:, b, :], t2[:], AF.Relu)

            # ================= Layers 2,3 =================================
            for li, (w, bias, brow) in enumerate(
                    [(w1, b1, brow1), (w2, b2, brow2)]):
                for b in range(NT):
                    ht = h_sb[:, b, :]
                    sums = sp.tile([128, 1], F32, tag="sums")
                    nc.vector.tensor_reduce(sums[:], ht, AX, OP.add)
                    negmu = sp.tile([128, 1], F32, tag="negmu")
                    nc.vector.tensor_scalar_mul(negmu[:], sums[:], -1.0 / D_H)
                    sq = wp.tile([128, D_H], F32, tag="sq")
                    ssq = sp.tile([128, 1], F32, tag="ssq")
                    nc.scalar.activation(sq[:], ht, AF.Square, bias=negmu[:],
                                         accum_out=ssq[:])
                    varp = sp.tile([128, 1], F32, tag="varp")
                    nc.vector.tensor_scalar(varp[:], ssq[:], 1.0 / D_H, EPS,
                                            OP.mult, OP.add)
                    sd = sp.tile([128, 1], F32, tag="sd")
                    nc.scalar.sqrt(sd[:], varp[:])
                    rstd = sp.tile([128, 1], F32, tag="rstd")
                    nc.vector.reciprocal(rstd[:], sd[:])
                    s = sp.tile([128, 1], F32, tag="s")
                    nc.vector.tensor_tensor(s[:], rstd[:], dv[:, b:b + 1],
                                            OP.mult)
                    z = wp.tile([128, D_H], BF, tag="z")
                    nc.vector.tensor_scalar(z[:], ht, negmu[:], s[:],
                                            OP.add, OP.mult)
                    mm = transpose_mm(z, w, D_H, 2)
                    u = wp.tile([128, D_H], BF, tag="u")
                    if brow is not None:
                        nc.vector.scalar_tensor_tensor(
                            u[:], brow[:], dv[:, b:b + 1], mm[:],
                            OP.mult, OP.add)
                    else:
                        nc.scalar.activation(u[:], mm[:], AF.Copy)
                    nc.sync.dma_start(r3(ul23, D_H)[:, b, :], u[:])
                    nc.sync.dma_start(r3(VA23[0:NPAD], D_H)[:, b, :], u[:])
                zero_plane(VB23, D_H, zbf)
                allgather(ul23, uf23)
                aggregate(uf23, VA23, VB23, D_H, BF)
                for b in range(NT):
                    vA = wp.tile([128, D_H], BF, tag="vA")
                    nc.sync.dma_start(vA[:], r3(VA23[0:NPAD], D_H)[:, b, :])
                    vB = wp.tile([128, D_H], BF, tag="vB")
                    nc.sync.dma_start(vB[:], r3(VB23[0:NPAD], D_H)[:, b, :])
                    t = wp.tile([128, D_H], F32, tag="t")
                    nc.vector.scalar_tensor_tensor(
                        t[:], vB[:], dv[:, b:b + 1], bias[:], OP.mult, OP.add)
                    t2 = wp.tile([128, D_H], F32, tag="t2")
                    nc.vector.scalar_tensor_tensor(
                        t2[:], vA[:], dv[:, b:b + 1], t[:], OP.mult, OP.add)
                    r = wp.tile([128, D_H], F32, tag="r")
                    nc.scalar.activation(r[:], t2[:], AF.Relu)
                    nc.vector.tensor_tensor(h_sb[:, b, :], r[:],
                                            h_sb[:, b, :], OP.add)

            # ================= Layer 4 ====================================
            for b in range(NT):
                z = wp.tile([128, D_H], BF, tag="z")
                nc.vector.tensor_scalar_mul(z[:], h_sb[:, b, :],
                                            dv[:, b:b + 1])
                mm = transpose_mm(z, w3, D_OUT, 2)
                u = wp.tile([128, D_OUT], F32, tag="u")
                nc.scalar.activation(u[:], mm[:], AF.Copy)
                nc.sync.dma_start(r3(ul4, D_OUT)[:, b, :], u[:])
                nc.sync.dma_start(r3(VA4[0:NPAD], D_OUT)[:, b, :], u[:])
            zero_plane(VB4, D_OUT, zf)
            allgather(ul4, uf4)
            aggregate(uf4, VA4, VB4, D_OUT, F32)
            for b in range(NT):
                vA = wp.tile([128, D_OUT], F32, tag="vA")
                nc.sync.dma_start(vA[:], r3(VA4[0:NPAD], D_OUT)[:, b, :])
                vB = wp.tile([128, D_OUT], F32, tag="vB")
                nc.sync.dma_start(vB[:], r3(VB4[0:NPAD], D_OUT)[:, b, :])
                y = wp.tile([128, D_OUT], F32, tag="t")
                nc.vector.scalar_tensor_tensor(
                    y[:], vB[:], dv[:, b:b + 1], b3[:], OP.mult, OP.add)
                y2 = wp.tile([128, D_OUT], F32, tag="t2")
                nc.vector.scalar_tensor_tensor(
                    y2[:], vA[:], dv[:, b:b + 1], y[:], OP.mult, OP.add)
                sums = sp.tile([128, 1], F32, tag="sums")
                nc.vector.tensor_reduce(sums[:], y2[:], AX, OP.add)
                negmu = sp.tile([128, 1], F32, tag="negmu")
                nc.vector.tensor_scalar_mul(negmu[:], sums[:], -1.0 / D_OUT)
                sq = wp.tile([128, D_OUT], F32, tag="sq")
                ssq = sp.tile([128, 1], F32, tag="ssq")
                nc.scalar.activation(sq[:], y2[:], AF.Square, bias=negmu[:],
                                     accum_out=ssq[:])
                varp = sp.tile([128, 1], F32, tag="varp")
                nc.vector.tensor_scalar(varp[:], ssq[:], 1.0 / D_OUT, EPS,
                                        OP.mult, OP.add)
                sd = sp.tile([128, 1], F32, tag="sd")
                nc.scalar.sqrt(sd[:], varp[:])
                rstd = sp.tile([128, 1], F32, tag="rstd")
                nc.vector.reciprocal(rstd[:], sd[:])
                zo = wp.tile([128, D_OUT], F32, tag="r")
                nc.vector.tensor_scalar(zo[:], y2[:], negmu[:], rstd[:],
                                        OP.add, OP.mult)
                if fg is not None:
                    zo2 = wp.tile([128, D_OUT], F32, tag="zo2")
                    nc.vector.tensor_tensor(zo2[:], zo[:], fg[:], OP.mult)
                    zo = zo2
                if fb is not None:
                    zo3 = wp.tile([128, D_OUT], F32, tag="zo3")
                    nc.vector.tensor_tensor(zo3[:], zo[:], fb[:], OP.add)
                    zo = zo3
                lo = b * 128
                nrow = min(128, NLOC - lo)
                if nrow > 0:
                    nc.sync.dma_start(out_p[lo:lo + nrow, :], zo[0:nrow, :])

    nc.compile()
    return nc


_CACHE = {}


def kernel(x, edge_index, W0, b0, W1, b1, W2, b2, W3, b3,
           ln0_g, ln0_b, ln1_g, ln1_b, fln_g, fln_b):
    x = np.asarray(x, np.float32)
    edge_index = np.asarray(edge_index)
    gidx, sidx, schedule, M, dinv = _prep_graph(edge_index)

    W1f = np.asarray(ln0_g, np.float32)[:, None] * np.asarray(W1, np.float32)
    W2f = np.asarray(ln1_g, np.float32)[:, None] * np.asarray(W2, np.float32)
    brow1 = np.asarray(ln0_b, np.float32) @ np.asarray(W1, np.float32)
    brow2 = np.asarray(ln1_b, np.float32) @ np.asarray(W2, np.float32)
    use_brow1 = bool(np.any(brow1 != 0))
    use_brow2 = bool(np.any(brow2 != 0))
    use_fg = bool(np.any(np.asarray(fln_g) != 1))
    use_fb = bool(np.any(np.asarray(fln_b) != 0))

    key = (M, tuple(schedule), use_brow1, use_brow2, use_fg, use_fb)
    if key not in _CACHE:
        _CACHE[key] = _build(M, schedule, use_brow1, use_brow2,
                             use_fg, use_fb)
    nc = _CACHE[key]

    u0 = _pad_nodes(dinv[:, None].astype(np.float32) * x)
    dinv_pad = np.zeros((NCORES, NPAD), np.float32)
    for r in range(NCORES):
        dinv_pad[r, :NLOC] = dinv[r * NLOC:(r + 1) * NLOC]

    def chunk2(Wf):
        return np.stack([Wf[0:128], Wf[128:256]]).astype(BF16)

    common = {
        "ident": np.eye(128, dtype=BF16),
        "w0": np.asarray(W0, np.float32).astype(BF16),
        "w1": chunk2(W1f), "w2": chunk2(W2f),
        "w3": chunk2(np.asarray(W3, np.float32)),
        "b0r": _rep(b0), "b1r": _rep(b1), "b2r": _rep(b2), "b3r": _rep(b3),
    }
    if use_brow1:
        common["brow1r"] = _rep(brow1)
    if use_brow2:
        common["brow2r"] = _rep(brow2)
    if use_fg:
        common["fgr"] = _rep(fln_g)
    if use_fb:
        common["fbr"] = _rep(fln_b)

    in_maps = []
    for r in range(NCORES):
        m = dict(common)
        m["u0_own"] = u0[r * NPAD:(r + 1) * NPAD]
        m["gidx"] = gidx[r]
        m["sidx"] = sidx[r]
        m["dinv"] = np.ascontiguousarray(dinv_pad[r].reshape(NT, 128).T)
        in_maps.append(m)

    res = run_bass_kernel_spmd(nc, in_maps, core_ids=list(range(NCORES)))
    out = np.concatenate([res.results[r]["out"] for r in range(NCORES)],
                         axis=0)
    return out.astype(np.float32)
